# revision 4
# baseline (speedup 1.0000x reference)
"""Trainium2 Bass kernel: per-batch cosine-distance matrix.

out[b] = 1 - metric[b] @ metric[b].T   where metric = x / ||x||_2 (last dim)
x: [32, 1024, 768] f32  ->  out: [32, 1024, 1024] f32

Sharding: data-parallel over batch. 8 cores x 4 batches each; no
cross-core communication. Per core, per batch:
  1. DMA x tiles [128, 768] f32 (8 t-tiles)
  2. ACT Square+accum -> row sum-of-squares; DVE reciprocal; ACT sqrt
     -> rinv = 1/||x_t||
  3. ACT copy with per-partition scale -> metric tile, cast to bf16
  4. xbar DMA transpose (SBUF->SBUF) -> metricT [768, 1024] bf16
  5. 96 bf16 matmuls (M=128, N=512, K=128 x 6) accumulating in PSUM
  6. DVE tensor_scalar (x*-1+1) PSUM -> SBUF f32 = 1 - scores
  7. DMA out [128, 1024] f32 rows
"""

import sys
from contextlib import ExitStack

_TRN_REPO = "/opt/trn_rl_repo"
if _TRN_REPO not in sys.path:
    sys.path.insert(0, _TRN_REPO)

import numpy as np

import concourse.bacc as bacc
import concourse.mybir as mybir
import concourse.tile as tile
from concourse.bass_utils import run_bass_kernel_spmd

B, T, C = 32, 1024, 768
N_CORES = 8
BPC = B // N_CORES  # batches per core
KC = C // 128       # 6 k-chunks
TT = T // 128       # 8 t-tiles
F32 = mybir.dt.float32
BF16 = mybir.dt.bfloat16
AF = mybir.ActivationFunctionType
ALU = mybir.AluOpType


def build():
    nc = bacc.Bacc("TRN2", target_bir_lowering=False, debug=False,
                   num_devices=N_CORES)
    x = nc.dram_tensor("x", [BPC, T, C], F32, kind="ExternalInput").ap()
    out = nc.dram_tensor("out", [BPC, T, T], F32, kind="ExternalOutput").ap()

    with tile.TileContext(nc) as tc, ExitStack() as ctx:
        x_pool = ctx.enter_context(tc.tile_pool(name="x", bufs=3))
        sq_pool = ctx.enter_context(tc.tile_pool(name="sq", bufs=1))
        s_pool = ctx.enter_context(tc.tile_pool(name="s", bufs=8))
        mt_pool = ctx.enter_context(tc.tile_pool(name="mt", bufs=4))
        mT_pool = ctx.enter_context(tc.tile_pool(name="mT", bufs=2))
        ob_pool = ctx.enter_context(tc.tile_pool(name="ob", bufs=4))
        mm_pool = ctx.enter_context(
            tc.tile_pool(name="mm", bufs=6, space="PSUM"))

        for b in range(BPC):
            # --- normalize rows, cast to bf16, transpose via xbar DMA ---
            mT = mT_pool.tile([128, KC * T], BF16)
            mT3 = mT[:].rearrange("p (k t) -> p k t", k=KC)
            for i in range(TT):
                xt = x_pool.tile([128, C], F32)
                nc.sync.dma_start(xt[:], x[b, i * 128:(i + 1) * 128, :])
                sq = sq_pool.tile([128, C], F32, tag="sq")
                ss = s_pool.tile([128, 1], F32, tag="ss")
                nc.scalar.activation(sq[:], xt[:], AF.Square,
                                     accum_out=ss[:])
                rr = s_pool.tile([128, 1], F32, tag="rr")
                nc.vector.reciprocal(rr[:], ss[:])
                rs = s_pool.tile([128, 1], F32, tag="rs")
                nc.scalar.sqrt(rs[:], rr[:])
                mt = mt_pool.tile([128, C], BF16)
                nc.scalar.activation(mt[:], xt[:], AF.Copy, bias=0.0,
                                     scale=rs[:])
                nc.scalar.dma_start_transpose(
                    mT3[:, :, i * 128:(i + 1) * 128], mt[:])

            # --- Gram matmuls + 1-x + store ---
            for m in range(TT):
                ob = ob_pool.tile([128, T], F32)
                for nh in range(2):
                    ps = mm_pool.tile([128, 512], F32)
                    for k in range(KC):
                        nc.tensor.matmul(
                            ps[:],
                            mT[:, k * T + m * 128: k * T + (m + 1) * 128],
                            mT[:, k * T + nh * 512: k * T + nh * 512 + 512],
                            start=(k == 0), stop=(k == KC - 1))
                    nc.vector.tensor_scalar(
                        ob[:, nh * 512:(nh + 1) * 512], ps[:],
                        -1.0, 1.0, ALU.mult, ALU.add)
                nc.sync.dma_start(out[b, m * 128:(m + 1) * 128, :], ob[:])

    nc.compile()
    return nc


def run(x, trace=False):
    nc = build()
    x = np.ascontiguousarray(np.asarray(x, dtype=np.float32))
    in_maps = [{"x": x[i * BPC:(i + 1) * BPC]} for i in range(N_CORES)]
    res = run_bass_kernel_spmd(nc, in_maps, list(range(N_CORES)),
                               trace=trace)
    out = np.concatenate([res.results[i]["out"] for i in range(N_CORES)],
                         axis=0)
    return out, res


def kernel(x):
    out, _ = run(x, trace=False)
    return out


# revision 7
# speedup vs baseline: 1.9080x; 1.9080x over previous
"""Trainium2 Bass kernel: per-batch cosine-distance matrix.

out[b] = 1 - metric[b] @ metric[b].T   where metric = x / ||x||_2 (last dim)
x: [32, 1024, 768] f32  ->  out: [32, 1024, 1024] f32

Sharding: data-parallel over batch. 8 cores x 4 batches each; no
cross-core communication. Per core, per batch:
  1. DMA x tiles [128, 768] f32 (8 t-tiles)
  2. ACT Square+accum -> row sum-of-squares; DVE reciprocal; ACT sqrt
     -> rinv = 1/||x_t||
  3. ACT copy with per-partition scale -> metric tile, cast to bf16
  4. xbar DMA transpose (SBUF->SBUF) -> metricT [768, 1024] bf16
  5. 96 bf16 matmuls (M=128, N=512, K=128 x 6) accumulating in PSUM
  6. DVE tensor_scalar (x*-1+1) PSUM -> SBUF f32 = 1 - scores
  7. DMA out [128, 1024] f32 rows
"""

import sys
from contextlib import ExitStack

_TRN_REPO = "/opt/trn_rl_repo"
if _TRN_REPO not in sys.path:
    sys.path.insert(0, _TRN_REPO)

import numpy as np

import concourse.bacc as bacc
import concourse.mybir as mybir
import concourse.tile as tile
from concourse.bass_utils import run_bass_kernel_spmd
from concourse.masks import make_identity

B, T, C = 32, 1024, 768
N_CORES = 8
BPC = B // N_CORES  # batches per core
KC = C // 128       # 6 k-chunks
TT = T // 128       # 8 t-tiles
F32 = mybir.dt.float32
BF16 = mybir.dt.bfloat16
AF = mybir.ActivationFunctionType
ALU = mybir.AluOpType


def build():
    nc = bacc.Bacc("TRN2", target_bir_lowering=False, debug=False,
                   num_devices=N_CORES)
    x = nc.dram_tensor("x", [BPC, T, C], F32, kind="ExternalInput").ap()
    out = nc.dram_tensor("out", [BPC, T, T], F32, kind="ExternalOutput").ap()

    with tile.TileContext(nc) as tc, ExitStack() as ctx:
        x_pool = ctx.enter_context(tc.tile_pool(name="x", bufs=3))
        sq_pool = ctx.enter_context(tc.tile_pool(name="sq", bufs=1))
        s_pool = ctx.enter_context(tc.tile_pool(name="s", bufs=8))
        mt_pool = ctx.enter_context(tc.tile_pool(name="mt", bufs=12))
        mT_pool = ctx.enter_context(tc.tile_pool(name="mT", bufs=2))
        ob_pool = ctx.enter_context(tc.tile_pool(name="ob", bufs=4))
        ident_pool = ctx.enter_context(tc.tile_pool(name="ident", bufs=1))
        tp_pool = ctx.enter_context(
            tc.tile_pool(name="tp", bufs=2, space="PSUM"))
        mm_pool = ctx.enter_context(
            tc.tile_pool(name="mm", bufs=6, space="PSUM"))

        ident = ident_pool.tile([128, 128], BF16)
        make_identity(nc, ident[:])

        for b in range(BPC):
            # --- normalize rows, cast to bf16 ---
            mts = []
            for i in range(TT):
                xt = x_pool.tile([128, C], F32)
                nc.sync.dma_start(xt[:], x[b, i * 128:(i + 1) * 128, :])
                sq = sq_pool.tile([128, C], F32, tag="sq")
                ss = s_pool.tile([128, 1], F32, tag="ss")
                nc.scalar.activation(sq[:], xt[:], AF.Square,
                                     accum_out=ss[:])
                rr = s_pool.tile([128, 1], F32, tag="rr")
                nc.vector.reciprocal(rr[:], ss[:])
                rs = s_pool.tile([128, 1], F32, tag="rs")
                nc.scalar.sqrt(rs[:], rr[:])
                mt = mt_pool.tile([128, C], BF16)
                nc.scalar.activation(mt[:], xt[:], AF.Copy, bias=0.0,
                                     scale=rs[:])
                mts.append(mt)

            # --- transpose to metricT [C, T] bf16 via PE ---
            mT = mT_pool.tile([128, KC * T], BF16)
            for k in range(KC):
                tp = tp_pool.tile([128, T], BF16)
                for i in range(TT):
                    nc.tensor.transpose(tp[:, i * 128:(i + 1) * 128],
                                        mts[i][:, k * 128:(k + 1) * 128],
                                        ident[:])
                nc.vector.tensor_copy(mT[:, k * T:(k + 1) * T], tp[:])

            # --- Gram matmuls + 1-x + store ---
            for m in range(TT):
                ob = ob_pool.tile([128, T], F32)
                for nh in range(2):
                    ps = mm_pool.tile([128, 512], F32)
                    for k in range(KC):
                        nc.tensor.matmul(
                            ps[:],
                            mT[:, k * T + m * 128: k * T + (m + 1) * 128],
                            mT[:, k * T + nh * 512: k * T + nh * 512 + 512],
                            start=(k == 0), stop=(k == KC - 1))
                    nc.vector.tensor_scalar(
                        ob[:, nh * 512:(nh + 1) * 512], ps[:],
                        -1.0, 1.0, ALU.mult, ALU.add)
                nc.sync.dma_start(out[b, m * 128:(m + 1) * 128, :], ob[:])

    nc.compile()
    return nc


def run(x, trace=False):
    nc = build()
    x = np.ascontiguousarray(np.asarray(x, dtype=np.float32))
    in_maps = [{"x": x[i * BPC:(i + 1) * BPC]} for i in range(N_CORES)]
    res = run_bass_kernel_spmd(nc, in_maps, list(range(N_CORES)),
                               trace=trace)
    out = np.concatenate([res.results[i]["out"] for i in range(N_CORES)],
                         axis=0)
    return out, res


def kernel(x):
    out, _ = run(x, trace=False)
    return out


# revision 21
# speedup vs baseline: 1.9860x; 1.0409x over previous
"""Trainium2 Bass kernel: per-batch cosine-distance matrix.

out[b] = 1 - metric[b] @ metric[b].T   where metric = x / ||x||_2 (last dim)
x: [32, 1024, 768] f32  ->  out: [32, 1024, 1024] f32

Sharding: data-parallel over batch. 8 cores x 4 batches each; no
cross-core communication. Per core, per batch:
  1. DMA x tiles [128, 768] f32 (8 t-tiles)
  2. ACT Square+accum -> row sum-of-squares; DVE reciprocal; ACT sqrt
     -> rinv = 1/||x_t||
  3. ACT copy with per-partition scale -> metric tile, cast to bf16
  4. xbar DMA transpose (SBUF->SBUF) -> metricT [768, 1024] bf16
  5. 96 bf16 matmuls (M=128, N=512, K=128 x 6) accumulating in PSUM
  6. DVE tensor_scalar (x*-1+1) PSUM -> SBUF f32 = 1 - scores
  7. DMA out [128, 1024] f32 rows
"""

import sys
from contextlib import ExitStack

_TRN_REPO = "/opt/trn_rl_repo"
if _TRN_REPO not in sys.path:
    sys.path.insert(0, _TRN_REPO)

import numpy as np

import concourse.bacc as bacc
import concourse.mybir as mybir
import concourse.tile as tile
from concourse.bass_utils import run_bass_kernel_spmd
from concourse.masks import make_identity

B, T, C = 32, 1024, 768
N_CORES = 8
BPC = B // N_CORES  # batches per core
KC = C // 128       # 6 k-chunks
TT = T // 128       # 8 t-tiles
F32 = mybir.dt.float32
BF16 = mybir.dt.bfloat16
AF = mybir.ActivationFunctionType
ALU = mybir.AluOpType


def build():
    nc = bacc.Bacc("TRN2", target_bir_lowering=False, debug=False,
                   num_devices=N_CORES)
    x = nc.dram_tensor("x", [BPC, T, C], F32, kind="ExternalInput").ap()
    out = nc.dram_tensor("out", [BPC, T, T], F32, kind="ExternalOutput").ap()

    with tile.TileContext(nc) as tc, ExitStack() as ctx:
        x_pool = ctx.enter_context(tc.tile_pool(name="x", bufs=6))
        sq_pool = ctx.enter_context(tc.tile_pool(name="sq", bufs=1))
        s_pool = ctx.enter_context(tc.tile_pool(name="s", bufs=8))
        mt_pool = ctx.enter_context(tc.tile_pool(name="mt", bufs=12))
        mT_pool = ctx.enter_context(tc.tile_pool(name="mT", bufs=2))
        ob_pool = ctx.enter_context(tc.tile_pool(name="ob", bufs=4))
        ident_pool = ctx.enter_context(tc.tile_pool(name="ident", bufs=1))
        tp_pool = ctx.enter_context(
            tc.tile_pool(name="tp", bufs=3, space="PSUM"))
        mm_pool = ctx.enter_context(
            tc.tile_pool(name="mm", bufs=5, space="PSUM"))

        ident = ident_pool.tile([128, 128], BF16)
        make_identity(nc, ident[:])

        # warm the ACT tables (Square, Sqrt) while the first DMAs fly
        warm = s_pool.tile([128, 1], F32, tag="warm")
        nc.vector.memset(warm[:], 1.0)
        warm2 = s_pool.tile([128, 1], F32, tag="warm2")
        nc.scalar.square(warm2[:], warm[:])
        nc.scalar.sqrt(warm2[:], warm[:])

        def emit_normalize(b):
            # batch 0's chain gates PE start: put its scales on DVE so the
            # ACT chain is short during the fill.
            mts = []
            for i in range(TT):
                xt = x_pool.tile([128, C], F32, tag="xt", name=f"xt_{b}_{i}")
                nc.sync.dma_start(xt[:], x[b, i * 128:(i + 1) * 128, :])
                ss = s_pool.tile([128, 1], F32, tag="ss", name=f"ss_{b}_{i}")
                sq = sq_pool.tile([128, C], F32, tag="sq", name=f"sq_{b}_{i}")
                nc.scalar.activation(sq[:], xt[:], AF.Square,
                                     accum_out=ss[:])
                rr = s_pool.tile([128, 1], F32, tag="rr", name=f"rr_{b}_{i}")
                nc.vector.reciprocal(rr[:], ss[:])
                rs = s_pool.tile([128, 1], F32, tag="rs", name=f"rs_{b}_{i}")
                nc.scalar.sqrt(rs[:], rr[:])
                mt = mt_pool.tile([128, C], BF16, tag="mt", name=f"mt_{b}_{i}")
                if b == 0 or i >= TT - 2:
                    # DVE scales where the ACT chain latency gates PE work
                    nc.vector.tensor_scalar_mul(mt[:], xt[:], rs[:])
                else:
                    nc.scalar.activation(mt[:], xt[:], AF.Copy, bias=0.0,
                                         scale=rs[:])
                mts.append(mt)
            return mts

        def emit_transpose_i(b, mts, mT3, i):
            # all 6 chunk-transposes of t-tile i into one PSUM bank, then
            # one strided DVE copy into mT's per-chunk columns
            tp = tp_pool.tile([128, KC * 128], BF16, tag="tp",
                              name=f"tp_{b}_{i}")
            for k in range(KC):
                nc.tensor.transpose(tp[:, k * 128:(k + 1) * 128],
                                    mts[i][:, k * 128:(k + 1) * 128],
                                    ident[:])
            tp3 = tp[:].rearrange("p (k t) -> p k t", k=KC)
            nc.vector.tensor_copy(mT3[:, :, i * 128:(i + 1) * 128], tp3)

        def emit_quarter(b, mT, mq):
            # k-outer sweep over an m-pair: 4 open PSUM groups
            pss = [[mm_pool.tile([128, 512], F32, tag="ps",
                                 name=f"ps_{b}_{mq}_{m2}_{nh}")
                    for nh in range(2)] for m2 in range(2)]
            for k in range(KC):
                for m2 in range(2):
                    m = mq * 2 + m2
                    for nh in range(2):
                        nc.tensor.matmul(
                            pss[m2][nh][:],
                            mT[:, k * T + m * 128: k * T + (m + 1) * 128],
                            mT[:, k * T + nh * 512: k * T + nh * 512 + 512],
                            start=(k == 0), stop=(k == KC - 1))
            for m2 in range(2):
                m = mq * 2 + m2
                ob = ob_pool.tile([128, T], F32, tag="ob",
                                  name=f"ob_{b}_{mq}_{m2}")
                for nh in range(2):
                    nc.vector.tensor_scalar(
                        ob[:, nh * 512:(nh + 1) * 512], pss[m2][nh][:],
                        -1.0, 1.0, ALU.mult, ALU.add)
                nc.gpsimd.dma_start(out[b, m * 128:(m + 1) * 128, :],
                                    ob[:])

        # software-pipelined emission: batch b's normalize chain first, then
        # b-1's MM quarters interleaved with b's per-tile transposes, so
        # PE fills between quarters and DVE sees tp-copies before the next
        # quarter's output copies.
        prev = None  # (b, mT)
        for b in range(BPC):
            mts = emit_normalize(b)
            mT = mT_pool.tile([128, KC * T], BF16, tag="mT", name=f"mT_{b}")
            mT3 = mT[:].rearrange("p (k t) -> p k t", k=KC)
            if prev is None:
                for i in range(TT):
                    emit_transpose_i(b, mts, mT3, i)
            else:
                sched = ([0, 1, 2], [3, 4, 5], [6, 7], [])
                for mq in range(TT // 2):
                    emit_quarter(*prev, mq)
                    for i in sched[mq]:
                        emit_transpose_i(b, mts, mT3, i)
            prev = (b, mT)
        for mq in range(TT // 2):
            emit_quarter(*prev, mq)

    nc.compile()
    return nc


def run(x, trace=False):
    nc = build()
    x = np.ascontiguousarray(np.asarray(x, dtype=np.float32))
    in_maps = [{"x": x[i * BPC:(i + 1) * BPC]} for i in range(N_CORES)]
    res = run_bass_kernel_spmd(nc, in_maps, list(range(N_CORES)),
                               trace=trace)
    out = np.concatenate([res.results[i]["out"] for i in range(N_CORES)],
                         axis=0)
    return out, res


def kernel(x):
    out, _ = run(x, trace=False)
    return out


# revision 23
# speedup vs baseline: 2.0018x; 1.0079x over previous
"""Trainium2 Bass kernel: per-batch cosine-distance matrix.

out[b] = 1 - metric[b] @ metric[b].T   where metric = x / ||x||_2 (last dim)
x: [32, 1024, 768] f32  ->  out: [32, 1024, 1024] f32

Sharding: data-parallel over batch. 8 cores x 4 batches each; no
cross-core communication. Per core, per batch:
  1. DMA x tiles [128, 768] f32 (8 t-tiles)
  2. ACT Square+accum -> row sum-of-squares; DVE reciprocal; ACT sqrt
     -> rinv = 1/||x_t||
  3. ACT copy with per-partition scale -> metric tile, cast to bf16
  4. xbar DMA transpose (SBUF->SBUF) -> metricT [768, 1024] bf16
  5. 96 bf16 matmuls (M=128, N=512, K=128 x 6) accumulating in PSUM
  6. DVE tensor_scalar (x*-1+1) PSUM -> SBUF f32 = 1 - scores
  7. DMA out [128, 1024] f32 rows
"""

import sys
from contextlib import ExitStack

_TRN_REPO = "/opt/trn_rl_repo"
if _TRN_REPO not in sys.path:
    sys.path.insert(0, _TRN_REPO)

import numpy as np

import concourse.bacc as bacc
import concourse.mybir as mybir
import concourse.tile as tile
from concourse.bass_utils import run_bass_kernel_spmd
from concourse.masks import make_identity

B, T, C = 32, 1024, 768
N_CORES = 8
BPC = B // N_CORES  # batches per core
KC = C // 128       # 6 k-chunks
TT = T // 128       # 8 t-tiles
F32 = mybir.dt.float32
BF16 = mybir.dt.bfloat16
AF = mybir.ActivationFunctionType
ALU = mybir.AluOpType


def build():
    nc = bacc.Bacc("TRN2", target_bir_lowering=False, debug=False,
                   num_devices=N_CORES)
    x = nc.dram_tensor("x", [BPC, T, C], F32, kind="ExternalInput").ap()
    out = nc.dram_tensor("out", [BPC, T, T], F32, kind="ExternalOutput").ap()

    with tile.TileContext(nc) as tc, ExitStack() as ctx:
        x_pool = ctx.enter_context(tc.tile_pool(name="x", bufs=6))
        sq_pool = ctx.enter_context(tc.tile_pool(name="sq", bufs=1))
        s_pool = ctx.enter_context(tc.tile_pool(name="s", bufs=8))
        mt_pool = ctx.enter_context(tc.tile_pool(name="mt", bufs=12))
        mT_pool = ctx.enter_context(tc.tile_pool(name="mT", bufs=2))
        ob_pool = ctx.enter_context(tc.tile_pool(name="ob", bufs=4))
        ident_pool = ctx.enter_context(tc.tile_pool(name="ident", bufs=1))
        tp_pool = ctx.enter_context(
            tc.tile_pool(name="tp", bufs=3, space="PSUM"))
        mm_pool = ctx.enter_context(
            tc.tile_pool(name="mm", bufs=5, space="PSUM"))

        ident = ident_pool.tile([128, 128], BF16)
        make_identity(nc, ident[:])

        # warm the ACT tables (Square, Sqrt) while the first DMAs fly
        warm = s_pool.tile([128, 1], F32, tag="warm")
        nc.vector.memset(warm[:], 1.0)
        warm2 = s_pool.tile([128, 1], F32, tag="warm2")
        nc.scalar.square(warm2[:], warm[:])
        nc.scalar.sqrt(warm2[:], warm[:])

        def emit_normalize(b):
            # batch 0's chain gates PE start: put its scales on DVE so the
            # ACT chain is short during the fill.
            mts = []
            for i in range(TT):
                xt = x_pool.tile([128, C], F32, tag="xt", name=f"xt_{b}_{i}")
                nc.sync.dma_start(xt[:], x[b, i * 128:(i + 1) * 128, :])
                ss = s_pool.tile([128, 1], F32, tag="ss", name=f"ss_{b}_{i}")
                sq = sq_pool.tile([128, C], F32, tag="sq", name=f"sq_{b}_{i}")
                nc.scalar.activation(sq[:], xt[:], AF.Square,
                                     accum_out=ss[:])
                rr = s_pool.tile([128, 1], F32, tag="rr", name=f"rr_{b}_{i}")
                nc.vector.reciprocal(rr[:], ss[:])
                rs = s_pool.tile([128, 1], F32, tag="rs", name=f"rs_{b}_{i}")
                nc.scalar.sqrt(rs[:], rr[:])
                mt = mt_pool.tile([128, C], BF16, tag="mt", name=f"mt_{b}_{i}")
                if b == 0:
                    # fill phase: scales on DVE so the ACT chain is short
                    nc.vector.tensor_scalar_mul(mt[:], xt[:], rs[:])
                else:
                    nc.scalar.activation(mt[:], xt[:], AF.Copy, bias=0.0,
                                         scale=rs[:])
                mts.append(mt)
            return mts

        def emit_transpose_i(b, mts, mT3, i):
            # all 6 chunk-transposes of t-tile i into one PSUM bank, then
            # one strided DVE copy into mT's per-chunk columns
            tp = tp_pool.tile([128, KC * 128], BF16, tag="tp",
                              name=f"tp_{b}_{i}")
            for k in range(KC):
                nc.tensor.transpose(tp[:, k * 128:(k + 1) * 128],
                                    mts[i][:, k * 128:(k + 1) * 128],
                                    ident[:])
            tp3 = tp[:].rearrange("p (k t) -> p k t", k=KC)
            nc.vector.tensor_copy(mT3[:, :, i * 128:(i + 1) * 128], tp3)

        def emit_quarter(b, mT, mq):
            # k-outer sweep over an m-pair: 4 open PSUM groups
            pss = [[mm_pool.tile([128, 512], F32, tag="ps",
                                 name=f"ps_{b}_{mq}_{m2}_{nh}")
                    for nh in range(2)] for m2 in range(2)]
            for k in range(KC):
                for m2 in range(2):
                    m = mq * 2 + m2
                    for nh in range(2):
                        nc.tensor.matmul(
                            pss[m2][nh][:],
                            mT[:, k * T + m * 128: k * T + (m + 1) * 128],
                            mT[:, k * T + nh * 512: k * T + nh * 512 + 512],
                            start=(k == 0), stop=(k == KC - 1))
            for m2 in range(2):
                m = mq * 2 + m2
                ob = ob_pool.tile([128, T], F32, tag="ob",
                                  name=f"ob_{b}_{mq}_{m2}")
                for nh in range(2):
                    nc.vector.tensor_scalar(
                        ob[:, nh * 512:(nh + 1) * 512], pss[m2][nh][:],
                        -1.0, 1.0, ALU.mult, ALU.add)
                nc.gpsimd.dma_start(out[b, m * 128:(m + 1) * 128, :],
                                    ob[:])

        # software-pipelined emission: batch b's normalize chain first, then
        # b-1's MM quarters interleaved with b's per-tile transposes, so
        # PE fills between quarters and DVE sees tp-copies before the next
        # quarter's output copies.
        prev = None  # (b, mT)
        for b in range(BPC):
            mts = emit_normalize(b)
            mT = mT_pool.tile([128, KC * T], BF16, tag="mT", name=f"mT_{b}")
            mT3 = mT[:].rearrange("p (k t) -> p k t", k=KC)
            if prev is None:
                for i in range(TT):
                    emit_transpose_i(b, mts, mT3, i)
            else:
                for mq in range(TT // 2):
                    emit_quarter(*prev, mq)
                    emit_transpose_i(b, mts, mT3, 2 * mq)
                    emit_transpose_i(b, mts, mT3, 2 * mq + 1)
            prev = (b, mT)
        for mq in range(TT // 2):
            emit_quarter(*prev, mq)

    nc.compile()
    return nc


def run(x, trace=False):
    nc = build()
    x = np.ascontiguousarray(np.asarray(x, dtype=np.float32))
    in_maps = [{"x": x[i * BPC:(i + 1) * BPC]} for i in range(N_CORES)]
    res = run_bass_kernel_spmd(nc, in_maps, list(range(N_CORES)),
                               trace=trace)
    out = np.concatenate([res.results[i]["out"] for i in range(N_CORES)],
                         axis=0)
    return out, res


def kernel(x):
    out, _ = run(x, trace=False)
    return out


# revision 26
# speedup vs baseline: 2.3176x; 1.1578x over previous
"""Trainium2 Bass kernel: per-batch cosine-distance matrix.

out[b] = 1 - metric[b] @ metric[b].T   where metric = x / ||x||_2 (last dim)
x: [32, 1024, 768] f32  ->  out: [32, 1024, 1024] f32

Sharding: data-parallel over batch. 8 cores x 4 batches each; no
cross-core communication. Per core, per batch:
  1. DMA x tiles [128, 768] f32 (8 t-tiles)
  2. ACT Square+accum -> row sum-of-squares; DVE reciprocal; ACT sqrt
     -> rinv = 1/||x_t||
  3. ACT copy with per-partition scale -> metric tile, cast to bf16
  4. xbar DMA transpose (SBUF->SBUF) -> metricT [768, 1024] bf16
  5. 96 bf16 matmuls (M=128, N=512, K=128 x 6) accumulating in PSUM
  6. DVE tensor_scalar (x*-1+1) PSUM -> SBUF f32 = 1 - scores
  7. DMA out [128, 1024] f32 rows
"""

import sys
from contextlib import ExitStack

_TRN_REPO = "/opt/trn_rl_repo"
if _TRN_REPO not in sys.path:
    sys.path.insert(0, _TRN_REPO)

import numpy as np

import concourse.bacc as bacc
import concourse.mybir as mybir
import concourse.tile as tile
from concourse.bass_utils import run_bass_kernel_spmd
from concourse.masks import make_identity

B, T, C = 32, 1024, 768
N_CORES = 8
BPC = B // N_CORES  # batches per core
KC = C // 128       # 6 k-chunks
TT = T // 128       # 8 t-tiles
F32 = mybir.dt.float32
BF16 = mybir.dt.bfloat16
AF = mybir.ActivationFunctionType
ALU = mybir.AluOpType


def build():
    nc = bacc.Bacc("TRN2", target_bir_lowering=False, debug=False,
                   num_devices=N_CORES)
    x = nc.dram_tensor("x", [BPC, T, C], F32, kind="ExternalInput").ap()
    out = nc.dram_tensor("out", [BPC, T, T], F32, kind="ExternalOutput").ap()

    with tile.TileContext(nc) as tc, ExitStack() as ctx:
        x_pool = ctx.enter_context(tc.tile_pool(name="x", bufs=6))
        sq_pool = ctx.enter_context(tc.tile_pool(name="sq", bufs=1))
        s_pool = ctx.enter_context(tc.tile_pool(name="s", bufs=8))
        mt_pool = ctx.enter_context(tc.tile_pool(name="mt", bufs=12))
        mT_pool = ctx.enter_context(tc.tile_pool(name="mT", bufs=2))
        ob_pool = ctx.enter_context(tc.tile_pool(name="ob", bufs=12))
        ident_pool = ctx.enter_context(tc.tile_pool(name="ident", bufs=1))
        tp_pool = ctx.enter_context(
            tc.tile_pool(name="tp", bufs=3, space="PSUM"))
        mm_pool = ctx.enter_context(
            tc.tile_pool(name="mm", bufs=3, space="PSUM"))
        tpm_pool = ctx.enter_context(
            tc.tile_pool(name="tpm", bufs=2, space="PSUM"))

        ident = ident_pool.tile([128, 128], BF16)
        make_identity(nc, ident[:])
        identf = ident_pool.tile([128, 128], F32)
        make_identity(nc, identf[:])

        # warm the ACT tables (Square, Sqrt) while the first DMAs fly
        warm = s_pool.tile([128, 1], F32, tag="warm")
        nc.vector.memset(warm[:], 1.0)
        warm2 = s_pool.tile([128, 1], F32, tag="warm2")
        nc.scalar.square(warm2[:], warm[:])
        nc.scalar.sqrt(warm2[:], warm[:])

        def emit_normalize(b):
            # batch 0's chain gates PE start: put its scales on DVE so the
            # ACT chain is short during the fill.
            mts = []
            for i in range(TT):
                xt = x_pool.tile([128, C], F32, tag="xt", name=f"xt_{b}_{i}")
                nc.sync.dma_start(xt[:], x[b, i * 128:(i + 1) * 128, :])
                ss = s_pool.tile([128, 1], F32, tag="ss", name=f"ss_{b}_{i}")
                sq = sq_pool.tile([128, C], F32, tag="sq", name=f"sq_{b}_{i}")
                nc.scalar.activation(sq[:], xt[:], AF.Square,
                                     accum_out=ss[:])
                rr = s_pool.tile([128, 1], F32, tag="rr", name=f"rr_{b}_{i}")
                nc.vector.reciprocal(rr[:], ss[:])
                rs = s_pool.tile([128, 1], F32, tag="rs", name=f"rs_{b}_{i}")
                nc.scalar.sqrt(rs[:], rr[:])
                mt = mt_pool.tile([128, C], BF16, tag="mt", name=f"mt_{b}_{i}")
                if b == 0:
                    # fill phase: scales on DVE so the ACT chain is short
                    nc.vector.tensor_scalar_mul(mt[:], xt[:], rs[:])
                else:
                    nc.scalar.activation(mt[:], xt[:], AF.Copy, bias=0.0,
                                         scale=rs[:])
                mts.append(mt)
            return mts

        def emit_transpose_i(b, mts, mT3, i):
            # all 6 chunk-transposes of t-tile i into one PSUM bank, then
            # one strided DVE copy into mT's per-chunk columns
            tp = tp_pool.tile([128, KC * 128], BF16, tag="tp",
                              name=f"tp_{b}_{i}")
            for k in range(KC):
                nc.tensor.transpose(tp[:, k * 128:(k + 1) * 128],
                                    mts[i][:, k * 128:(k + 1) * 128],
                                    ident[:])
            tp3 = tp[:].rearrange("p (k t) -> p k t", k=KC)
            nc.vector.tensor_copy(mT3[:, :, i * 128:(i + 1) * 128], tp3)

        def emit_row(b, mT, obs, bm):
            # row bm: compute upper-triangle blocks (s >= bm*128) via MMs,
            # fill the lower part by transposing already-computed upper
            # blocks of earlier rows (output is symmetric).
            n0 = bm * 128
            W = T - n0
            ob = ob_pool.tile([128, T], F32, tag="ob", name=f"ob_{b}_{bm}")
            off = 0
            while off < W:
                w = min(512, W - off)
                ps = mm_pool.tile([128, w], F32, tag="ps",
                                  name=f"ps_{b}_{bm}_{off}")
                for k in range(KC):
                    nc.tensor.matmul(
                        ps[:],
                        mT[:, k * T + n0: k * T + n0 + 128],
                        mT[:, k * T + n0 + off: k * T + n0 + off + w],
                        start=(k == 0), stop=(k == KC - 1))
                nc.vector.tensor_scalar(
                    ob[:, n0 + off: n0 + off + w], ps[:],
                    -1.0, 1.0, ALU.mult, ALU.add)
                off += w
            g0 = 0
            while g0 < bm:
                gn = min(4, bm - g0)
                tpm = tpm_pool.tile([128, gn * 128], F32, tag="tpm",
                                    name=f"tpm_{b}_{bm}_{g0}")
                for j in range(gn):
                    bn = g0 + j
                    nc.tensor.transpose(tpm[:, j * 128:(j + 1) * 128],
                                        obs[bn][:, n0:n0 + 128], identf[:])
                nc.vector.tensor_copy(ob[:, g0 * 128:(g0 + gn) * 128],
                                      tpm[:])
                g0 += gn
            nc.gpsimd.dma_start(out[b, n0:n0 + 128, :], ob[:])
            obs.append(ob)

        # software-pipelined emission: batch b's normalize chain first, then
        # b-1's output rows interleaved with b's per-tile transposes.
        prev = None  # (b, mT, obs)
        for b in range(BPC):
            mts = emit_normalize(b)
            mT = mT_pool.tile([128, KC * T], BF16, tag="mT", name=f"mT_{b}")
            mT3 = mT[:].rearrange("p (k t) -> p k t", k=KC)
            if prev is None:
                for i in range(TT):
                    emit_transpose_i(b, mts, mT3, i)
            else:
                for bm in range(TT):
                    emit_row(*prev, bm)
                    emit_transpose_i(b, mts, mT3, bm)
            prev = (b, mT, [])
        for bm in range(TT):
            emit_row(*prev, bm)

    nc.compile()
    return nc


def run(x, trace=False):
    nc = build()
    x = np.ascontiguousarray(np.asarray(x, dtype=np.float32))
    in_maps = [{"x": x[i * BPC:(i + 1) * BPC]} for i in range(N_CORES)]
    res = run_bass_kernel_spmd(nc, in_maps, list(range(N_CORES)),
                               trace=trace)
    out = np.concatenate([res.results[i]["out"] for i in range(N_CORES)],
                         axis=0)
    return out, res


def kernel(x):
    out, _ = run(x, trace=False)
    return out


# revision 27
# speedup vs baseline: 2.4880x; 1.0735x over previous
"""Trainium2 Bass kernel: per-batch cosine-distance matrix.

out[b] = 1 - metric[b] @ metric[b].T   where metric = x / ||x||_2 (last dim)
x: [32, 1024, 768] f32  ->  out: [32, 1024, 1024] f32

Sharding: data-parallel over batch. 8 cores x 4 batches each; no
cross-core communication. Per core, per batch:
  1. DMA x tiles [128, 768] f32 (8 t-tiles)
  2. ACT Square+accum -> row sum-of-squares; DVE reciprocal; ACT sqrt
     -> rinv = 1/||x_t||
  3. ACT copy with per-partition scale -> metric tile, cast to bf16
  4. xbar DMA transpose (SBUF->SBUF) -> metricT [768, 1024] bf16
  5. 96 bf16 matmuls (M=128, N=512, K=128 x 6) accumulating in PSUM
  6. DVE tensor_scalar (x*-1+1) PSUM -> SBUF f32 = 1 - scores
  7. DMA out [128, 1024] f32 rows
"""

import sys
from contextlib import ExitStack

_TRN_REPO = "/opt/trn_rl_repo"
if _TRN_REPO not in sys.path:
    sys.path.insert(0, _TRN_REPO)

import numpy as np

import concourse.bacc as bacc
import concourse.mybir as mybir
import concourse.tile as tile
from concourse.bass_utils import run_bass_kernel_spmd
from concourse.masks import make_identity

B, T, C = 32, 1024, 768
N_CORES = 8
BPC = B // N_CORES  # batches per core
KC = C // 128       # 6 k-chunks
TT = T // 128       # 8 t-tiles
F32 = mybir.dt.float32
BF16 = mybir.dt.bfloat16
AF = mybir.ActivationFunctionType
ALU = mybir.AluOpType


def build():
    nc = bacc.Bacc("TRN2", target_bir_lowering=False, debug=False,
                   num_devices=N_CORES)
    x = nc.dram_tensor("x", [BPC, T, C], F32, kind="ExternalInput").ap()
    out = nc.dram_tensor("out", [BPC, T, T], F32, kind="ExternalOutput").ap()

    with tile.TileContext(nc) as tc, ExitStack() as ctx:
        x_pool = ctx.enter_context(tc.tile_pool(name="x", bufs=8))
        sq_pool = ctx.enter_context(tc.tile_pool(name="sq", bufs=1))
        s_pool = ctx.enter_context(tc.tile_pool(name="s", bufs=8))
        mt_pool = ctx.enter_context(tc.tile_pool(name="mt", bufs=16))
        mT_pool = ctx.enter_context(tc.tile_pool(name="mT", bufs=2))
        ob_pool = ctx.enter_context(tc.tile_pool(name="ob", bufs=12))
        ident_pool = ctx.enter_context(tc.tile_pool(name="ident", bufs=1))
        tp_pool = ctx.enter_context(
            tc.tile_pool(name="tp", bufs=3, space="PSUM"))
        mm_pool = ctx.enter_context(
            tc.tile_pool(name="mm", bufs=3, space="PSUM"))
        tpm_pool = ctx.enter_context(
            tc.tile_pool(name="tpm", bufs=2, space="PSUM"))

        ident = ident_pool.tile([128, 128], BF16)
        make_identity(nc, ident[:])
        identf = ident_pool.tile([128, 128], F32)
        make_identity(nc, identf[:])

        # warm the ACT tables (Square, Sqrt) while the first DMAs fly
        warm = s_pool.tile([128, 1], F32, tag="warm")
        nc.vector.memset(warm[:], 1.0)
        warm2 = s_pool.tile([128, 1], F32, tag="warm2")
        nc.scalar.square(warm2[:], warm[:])
        nc.scalar.sqrt(warm2[:], warm[:])

        def emit_normalize(b):
            # batch 0's chain gates PE start: put its scales on DVE so the
            # ACT chain is short during the fill.
            mts = []
            for i in range(TT):
                xt = x_pool.tile([128, C], F32, tag="xt", name=f"xt_{b}_{i}")
                nc.sync.dma_start(xt[:], x[b, i * 128:(i + 1) * 128, :])
                ss = s_pool.tile([128, 1], F32, tag="ss", name=f"ss_{b}_{i}")
                sq = sq_pool.tile([128, C], F32, tag="sq", name=f"sq_{b}_{i}")
                nc.scalar.activation(sq[:], xt[:], AF.Square,
                                     accum_out=ss[:])
                rr = s_pool.tile([128, 1], F32, tag="rr", name=f"rr_{b}_{i}")
                nc.vector.reciprocal(rr[:], ss[:])
                rs = s_pool.tile([128, 1], F32, tag="rs", name=f"rs_{b}_{i}")
                nc.scalar.sqrt(rs[:], rr[:])
                mt = mt_pool.tile([128, C], BF16, tag="mt", name=f"mt_{b}_{i}")
                if b == 0:
                    # fill phase: scales on DVE so the ACT chain is short
                    nc.vector.tensor_scalar_mul(mt[:], xt[:], rs[:])
                else:
                    nc.scalar.activation(mt[:], xt[:], AF.Copy, bias=0.0,
                                         scale=rs[:])
                mts.append(mt)
            return mts

        def emit_transpose_i(b, mts, mT3, i):
            # all 6 chunk-transposes of t-tile i into one PSUM bank, then
            # one strided DVE copy into mT's per-chunk columns
            tp = tp_pool.tile([128, KC * 128], BF16, tag="tp",
                              name=f"tp_{b}_{i}")
            for k in range(KC):
                nc.tensor.transpose(tp[:, k * 128:(k + 1) * 128],
                                    mts[i][:, k * 128:(k + 1) * 128],
                                    ident[:])
            tp3 = tp[:].rearrange("p (k t) -> p k t", k=KC)
            nc.vector.tensor_copy(mT3[:, :, i * 128:(i + 1) * 128], tp3)

        def emit_row(b, mT, obs, bm):
            # row bm: compute upper-triangle blocks (s >= bm*128) via MMs,
            # fill the lower part by transposing already-computed upper
            # blocks of earlier rows (output is symmetric).
            n0 = bm * 128
            W = T - n0
            ob = ob_pool.tile([128, T], F32, tag="ob", name=f"ob_{b}_{bm}")
            off = 0
            while off < W:
                w = min(512, W - off)
                ps = mm_pool.tile([128, w], F32, tag="ps",
                                  name=f"ps_{b}_{bm}_{off}")
                for k in range(KC):
                    nc.tensor.matmul(
                        ps[:],
                        mT[:, k * T + n0: k * T + n0 + 128],
                        mT[:, k * T + n0 + off: k * T + n0 + off + w],
                        start=(k == 0), stop=(k == KC - 1))
                nc.vector.tensor_scalar(
                    ob[:, n0 + off: n0 + off + w], ps[:],
                    -1.0, 1.0, ALU.mult, ALU.add)
                off += w
            g0 = 0
            while g0 < bm:
                gn = min(4, bm - g0)
                tpm = tpm_pool.tile([128, gn * 128], F32, tag="tpm",
                                    name=f"tpm_{b}_{bm}_{g0}")
                for j in range(gn):
                    bn = g0 + j
                    nc.tensor.transpose(tpm[:, j * 128:(j + 1) * 128],
                                        obs[bn][:, n0:n0 + 128], identf[:])
                nc.vector.tensor_copy(ob[:, g0 * 128:(g0 + gn) * 128],
                                      tpm[:])
                g0 += gn
            nc.gpsimd.dma_start(out[b, n0:n0 + 128, :], ob[:])
            obs.append(ob)

        # software-pipelined emission: batch b's normalize chain first, then
        # b-1's output rows interleaved with b's per-tile transposes.
        prev = None  # (b, mT, obs)
        for b in range(BPC):
            mts = emit_normalize(b)
            mT = mT_pool.tile([128, KC * T], BF16, tag="mT", name=f"mT_{b}")
            mT3 = mT[:].rearrange("p (k t) -> p k t", k=KC)
            if prev is None:
                for i in range(TT):
                    emit_transpose_i(b, mts, mT3, i)
            else:
                for bm in range(TT):
                    emit_row(*prev, bm)
                    emit_transpose_i(b, mts, mT3, bm)
            prev = (b, mT, [])
        for bm in range(TT):
            emit_row(*prev, bm)

    nc.compile()
    return nc


def run(x, trace=False):
    nc = build()
    x = np.ascontiguousarray(np.asarray(x, dtype=np.float32))
    in_maps = [{"x": x[i * BPC:(i + 1) * BPC]} for i in range(N_CORES)]
    res = run_bass_kernel_spmd(nc, in_maps, list(range(N_CORES)),
                               trace=trace)
    out = np.concatenate([res.results[i]["out"] for i in range(N_CORES)],
                         axis=0)
    return out, res


def kernel(x):
    out, _ = run(x, trace=False)
    return out
